# revision 8
# baseline (speedup 1.0000x reference)
"""Trainium2 Bass kernel for CausalSelfAttention (GQA + QK-RMSNorm + RoPE).

Problem shapes (hardcoded): B=2, T=2048, C=2048, n_head=16, n_kv_head=4,
head_dim=128. 8 NeuronCores: data-parallel over batch (2) x tensor-parallel
over kv-head groups (4). Core j handles batch j//4 and kv-head group j%4
(q heads 4*(j%4)..4*(j%4)+3). Each core computes a partial output projection
with its Wo row-slice; the host sums the 4 partials per batch (the unshard
step) and adds the bias.

On-device dtype is fp16 (inputs pre-cast on host) with fp32 PSUM accumulation.
Layout strategy (all matmuls contract over the partition dim):
  x^T via DMA-transpose from DRAM -> q/k/v projections in natural [T, d]
  layout -> RMSNorm (Square+accum on ACT, rsqrt = exp(-0.5*ln(ms)) to stay in
  one ACT table) + RoPE on DVE -> PE-transpose to q^T/k^T (fp16 PSUM) ->
  S^T = k^T' @ q^T per tk-chunk (tq range clipped at the causal diagonal) ->
  exp on ACT (PSUM->SBUF, bias -16*ln2 keeps P in fp16 range; the uniform
  scale cancels in the softmax normalization) -> causal mask via upper-tri
  multiply on the diagonal 128x128 block -> y_aug = P^T' @ [v | 1] accumulated
  over tk (the ones column yields the softmax denominators) -> divide via ACT
  copy scaled by reciprocal -> PE-transpose y -> partial out = y^T' @ Wo.
"""
import sys
import numpy as np

for _p in ("/opt/trn_rl_repo",):
    if _p not in sys.path:
        sys.path.insert(0, _p)

import concourse.bass as bass
import concourse.bacc as bacc
import concourse.mybir as mybir
import concourse.tile as tile
from concourse.bass_utils import run_bass_kernel_spmd

F32 = mybir.dt.float32
F16 = mybir.dt.float16
AF = mybir.ActivationFunctionType
ALU = mybir.AluOpType

B, T, C = 2, 2048, 2048
N_HEAD, N_KV_HEAD = 16, 4
HD = 128           # head dim
HALF = 64
G = N_HEAD // N_KV_HEAD      # 4 q heads per kv head = heads per core
NC_ = 8
TP = 4                       # tensor-parallel width (kv heads)
QCOLS = G * HD               # 512 q columns per core
SCALE = 1.0 / float(np.sqrt(HD))
EXP_BIAS = -16.0 * float(np.log(2.0))   # keep exp() outputs inside fp16 range
EPS = float(np.finfo(np.float32).eps)

_CACHE = {}


def build_nc(t=T):
    """Build the SPMD Tile kernel for sequence length t (t % 512 == 0)."""
    nt = t // 128          # number of 128-row T-chunks
    nb = t // 512          # number of 512-wide tq blocks
    ncc = C // 128         # number of 128 C-chunks

    nc = bacc.Bacc("TRN2", target_bir_lowering=False, debug=False,
                   num_devices=NC_)

    x = nc.dram_tensor("x", [t, C], F16, kind="ExternalInput")
    wq = nc.dram_tensor("wq", [C, QCOLS], F16, kind="ExternalInput")
    wkv = nc.dram_tensor("wkv", [C, 2 * HD], F16, kind="ExternalInput")
    wo = nc.dram_tensor("wo", [QCOLS, C], F16, kind="ExternalInput")
    cs5 = nc.dram_tensor("cos5", [t, 5 * HALF], F16, kind="ExternalInput")
    sn5 = nc.dram_tensor("sin5", [t, 5 * HALF], F16, kind="ExternalInput")
    mask01 = nc.dram_tensor("mask01", [128, 128], F16, kind="ExternalInput")
    ident = nc.dram_tensor("ident16", [128, 128], F16, kind="ExternalInput")
    out = nc.dram_tensor("out", [t, C], F16, kind="ExternalOutput")

    with tile.TileContext(nc) as tc:
        with (
            tc.tile_pool(name="const", bufs=1) as cpool,
            tc.tile_pool(name="wts", bufs=1) as wpool,
            tc.tile_pool(name="big", bufs=1) as bpool,
            tc.tile_pool(name="work", bufs=3) as work,
        ):
            # ------------- x^T first: DMA transposes on the sync queue -----
            xT = []
            for cc in range(ncc):
                xt = bpool.tile([128, t], F16, tag=f"xT{cc}", name=f"xT{cc}")
                nc.sync.dma_start_transpose(xt[:], x[:, cc * 128:(cc + 1) * 128])
                xT.append(xt)

            # ------------- constants / weights (ACT hwdge queue) -----------
            mask_t = cpool.tile([128, 128], F16, tag="mask")
            nc.scalar.dma_start(mask_t[:], mask01[:])
            id16 = cpool.tile([128, 128], F16, tag="id16")
            nc.scalar.dma_start(id16[:], ident[:])
            b_exp = cpool.tile([128, 1], F32, tag="bexp")
            nc.vector.memset(b_exp[:], EXP_BIAS)
            b_eps = cpool.tile([128, 1], F32, tag="beps")
            nc.vector.memset(b_eps[:], EPS)

            wq_t = wpool.tile([128, ncc, QCOLS], F16, tag="wq")
            nc.scalar.dma_start(
                wq_t[:], wq[:].rearrange("(cc p) n -> p cc n", p=128))
            wkv_t = wpool.tile([128, ncc, 2 * HD], F16, tag="wkv")
            nc.scalar.dma_start(
                wkv_t[:], wkv[:].rearrange("(cc p) n -> p cc n", p=128))
            wo_t = wpool.tile([128, G, C], F16, tag="wo")
            nc.scalar.dma_start(
                wo_t[:], wo[:].rearrange("(hc p) n -> p hc n", p=128))
            cos_t = wpool.tile([128, nt, 5 * HALF], F16, tag="cos")
            nc.scalar.dma_start(
                cos_t[:], cs5[:].rearrange("(i p) n -> p i n", p=128))
            sin_t = wpool.tile([128, nt, 5 * HALF], F16, tag="sin")
            nc.scalar.dma_start(
                sin_t[:], sn5[:].rearrange("(i p) n -> p i n", p=128))

            # persistent attention operands
            # qkT[:, h, :] = head h of q^T (h<4) or k^T (h==4)
            qkT = bpool.tile([128, 5, t], F16, tag="qkT")
            v_aug = bpool.tile([128, nt * (HD + 1)], F16, tag="vaug")
            nc.vector.memset(v_aug[:], 1.0)   # ones columns; data overwritten
            yT = [bpool.tile([128, t], F16, tag=f"yT{h}", name=f"yT{h}")
                  for h in range(G)]

            # ---------------- phase 1: projections + norm + rope ----------
            with tc.tile_pool(name="ps_proj", bufs=2, space="PSUM") as ps_proj:
              for i in range(nt):
                q_ps = ps_proj.tile([128, QCOLS], F32, tag="qps")
                kv_ps = ps_proj.tile([128, 2 * HD], F32, tag="kvps")
                for cc in range(ncc):
                    st = (cc == 0)
                    sp = (cc == ncc - 1)
                    nc.tensor.matmul(q_ps[:], xT[cc][:, i * 128:(i + 1) * 128],
                                     wq_t[:, cc, :], start=st, stop=sp)
                    nc.tensor.matmul(kv_ps[:], xT[cc][:, i * 128:(i + 1) * 128],
                                     wkv_t[:, cc, :], start=st, stop=sp)

                # rms stats for 4 q heads + 1 k head
                sumsq = work.tile([128, 5], F32, tag="sumsq")
                sq_scr = work.tile([128, 128], F16, tag="sqscr")
                for h in range(G):
                    nc.scalar.activation(sq_scr[:], q_ps[:, h * HD:(h + 1) * HD],
                                         AF.Square,
                                         accum_out=sumsq[:, h:h + 1])
                nc.scalar.activation(sq_scr[:], kv_ps[:, 0:HD], AF.Square,
                                     accum_out=sumsq[:, 4:5])
                lnms = work.tile([128, 5], F32, tag="lnms")
                nc.scalar.activation(lnms[:], sumsq[:], AF.Ln,
                                     bias=b_eps[:], scale=1.0 / HD)
                r5 = work.tile([128, 5], F32, tag="r5")
                nc.scalar.activation(r5[:], lnms[:], AF.Exp, scale=-0.5)

                # evacuate q/k with rms scale, v plain (fp16)
                qk_s = work.tile([128, 5 * HD], F16, tag="qks")
                for h in range(G):
                    nc.scalar.mul(qk_s[:, h * HD:(h + 1) * HD],
                                  q_ps[:, h * HD:(h + 1) * HD], r5[:, h:h + 1])
                nc.scalar.mul(qk_s[:, 4 * HD:5 * HD], kv_ps[:, 0:HD],
                              r5[:, 4:5])
                nc.scalar.copy(v_aug[:, i * (HD + 1):i * (HD + 1) + HD],
                               kv_ps[:, HD:2 * HD])

                # rope over 5 heads at once (strided views)
                qk_r = work.tile([128, 5 * HD], F16, tag="qkr")
                lo = qk_s[:].rearrange("p (h d) -> p h d", d=HD)[:, :, 0:HALF]
                hi = qk_s[:].rearrange("p (h d) -> p h d", d=HD)[:, :, HALF:HD]
                lo_o = qk_r[:].rearrange("p (h d) -> p h d", d=HD)[:, :, 0:HALF]
                hi_o = qk_r[:].rearrange("p (h d) -> p h d", d=HD)[:, :, HALF:HD]
                cos_v = cos_t[:, i, :].rearrange("p (h f) -> p h f", f=HALF)
                sin_v = sin_t[:, i, :].rearrange("p (h f) -> p h f", f=HALF)
                t1 = work.tile([128, 5 * HALF], F16, tag="t1")
                t2 = work.tile([128, 5 * HALF], F16, tag="t2")
                t1v = t1[:].rearrange("p (h f) -> p h f", f=HALF)
                t2v = t2[:].rearrange("p (h f) -> p h f", f=HALF)
                nc.vector.tensor_tensor(t1v, lo, cos_v, ALU.mult)
                nc.vector.tensor_tensor(t2v, hi, sin_v, ALU.mult)
                nc.vector.tensor_tensor(lo_o, t1v, t2v, ALU.subtract)
                nc.vector.tensor_tensor(t1v, lo, sin_v, ALU.mult)
                nc.vector.tensor_tensor(t2v, hi, cos_v, ALU.mult)
                nc.vector.tensor_tensor(hi_o, t1v, t2v, ALU.add)

                # PE-transpose the 5 roped heads -> fp16 PSUM -> DVE evac
                tps = ps_proj.tile([128, 5 * HD], F16, tag="tps")
                for h in range(5):
                    nc.tensor.transpose(tps[:, h * HD:(h + 1) * HD],
                                        qk_r[:, h * HD:(h + 1) * HD], id16[:])
                nc.vector.tensor_copy(
                    qkT[:, :, i * 128:(i + 1) * 128],
                    tps[:].rearrange("p (h d) -> p h d", d=HD))

            # ---------------- phases 2+3: attention + output projection ---
            with (
                tc.tile_pool(name="ps_s", bufs=2, space="PSUM") as ps_s,
                tc.tile_pool(name="ps_y", bufs=4, space="PSUM") as ps_y,
                tc.tile_pool(name="ps_t", bufs=2, space="PSUM") as ps_t,
            ):
              for h in range(G):
                for b in range(nb):
                    y_ps = [ps_y.tile([128, HD + 1], F32, tag="y", name=f"y{j}")
                            for j in range(4)]
                    nk = 4 * (b + 1)        # causal: tk chunks 0..nk-1
                    for k in range(nk):
                        # clip the tq range at the causal diagonal
                        tq0 = max(b * 512, k * 128)
                        w = (b + 1) * 512 - tq0
                        s_ps = ps_s.tile([128, 512], F32, tag="s")
                        nc.tensor.matmul(s_ps[:, 0:w],
                                         qkT[:, 4, k * 128:(k + 1) * 128],
                                         qkT[:, h, tq0:tq0 + w],
                                         start=True, stop=True)
                        pT = work.tile([128, 512], F16, tag="pT")
                        nc.scalar.activation(pT[:, 0:w], s_ps[:, 0:w], AF.Exp,
                                             bias=b_exp[:], scale=SCALE)
                        if k * 128 >= b * 512:   # diagonal block at offset 0
                            nc.vector.tensor_tensor(
                                pT[:, 0:128], pT[:, 0:128],
                                mask_t[:], ALU.mult)
                        for j in range(4):
                            tq = b * 4 + j
                            if tq * 128 < tq0:   # fully masked (tq < tk)
                                continue
                            off = tq * 128 - tq0
                            nc.tensor.matmul(
                                y_ps[j][:], pT[:, off:off + 128],
                                v_aug[:, k * (HD + 1):(k + 1) * (HD + 1)],
                                start=(k == 0), stop=(k == tq),
                                skip_group_check=True)
                    # divide by softmax sums, PE-transpose, DVE evac
                    ytp = ps_t.tile([128, 512], F16, tag="ytp")
                    for j in range(4):
                        rcp = work.tile([128, 1], F32, tag="rcp")
                        nc.vector.reciprocal(rcp[:], y_ps[j][:, HD:HD + 1])
                        y_sb = work.tile([128, HD], F16, tag="ysb")
                        nc.scalar.mul(y_sb[:], y_ps[j][:, 0:HD], rcp[:])
                        nc.tensor.transpose(ytp[:, j * 128:(j + 1) * 128],
                                            y_sb[:], id16[:])
                    nc.vector.tensor_copy(yT[h][:, b * 512:(b + 1) * 512],
                                          ytp[:])

              # -------------- output projection (partial) ------------------
              for tq in range(nt):
                for cb in range(C // 512):
                    o_ps = ps_s.tile([128, 512], F32, tag="s", name="o_ps")
                    for hc in range(G):
                        nc.tensor.matmul(o_ps[:],
                                         yT[hc][:, tq * 128:(tq + 1) * 128],
                                         wo_t[:, hc, cb * 512:(cb + 1) * 512],
                                         start=(hc == 0), stop=(hc == G - 1))
                    o_sb = work.tile([128, 512], F16, tag="osb")
                    if cb % 2 == 0:
                        nc.scalar.copy(o_sb[:], o_ps[:])
                    else:
                        nc.vector.tensor_copy(o_sb[:], o_ps[:])
                    nc.sync.dma_start(
                        out[tq * 128:(tq + 1) * 128, cb * 512:(cb + 1) * 512],
                        o_sb[:])
    nc.compile()
    return nc


def _prep_inputs(x, cos, sin, Wq, Wk, Wv, Wo, bo, t):
    """Build the 8 per-core input maps (host-side shard + fp16 cast)."""
    cos2 = np.asarray(cos, np.float32).reshape(-1, HALF)[:t]
    sin2 = np.asarray(sin, np.float32).reshape(-1, HALF)[:t]
    cos5 = np.tile(cos2, (1, 5)).astype(np.float16)
    sin5 = np.tile(sin2, (1, 5)).astype(np.float16)
    mask = np.triu(np.ones((128, 128), np.float16))  # [tk, tq]: 1 iff tq >= tk
    ident = np.eye(128, dtype=np.float16)
    Wq = np.asarray(Wq, np.float32)
    Wk = np.asarray(Wk, np.float32)
    Wv = np.asarray(Wv, np.float32)
    Wo = np.asarray(Wo, np.float32)
    x = np.asarray(x, np.float32)
    maps = []
    for core in range(NC_):
        b, tp = core // TP, core % TP
        wkv = np.concatenate(
            [Wk[:, tp * HD:(tp + 1) * HD], Wv[:, tp * HD:(tp + 1) * HD]],
            axis=1)
        maps.append({
            "x": x[b, :t].astype(np.float16),
            "wq": Wq[:, tp * QCOLS:(tp + 1) * QCOLS].astype(np.float16),
            "wkv": wkv.astype(np.float16),
            "wo": Wo[tp * QCOLS:(tp + 1) * QCOLS, :].astype(np.float16),
            "cos5": cos5, "sin5": sin5, "mask01": mask, "ident16": ident,
        })
    return maps


def run(x, cos, sin, Wq, Wk, Wv, Wo, bo, t=T, trace=False):
    key = t
    if key not in _CACHE:
        _CACHE[key] = build_nc(t)
    nc = _CACHE[key]
    maps = _prep_inputs(x, cos, sin, Wq, Wk, Wv, Wo, bo, t)
    res = run_bass_kernel_spmd(nc, maps, core_ids=list(range(NC_)),
                               trace=trace)
    bo = np.asarray(bo, np.float32)
    outp = np.empty((B, t, C), np.float32)
    for b in range(B):
        acc = res.results[b * TP]["out"].astype(np.float32)
        for tp in range(1, TP):
            acc += res.results[b * TP + tp]["out"].astype(np.float32)
        outp[b] = acc + bo[None, :]
    return outp, res


def kernel(x, cos, sin, Wq, Wk, Wv, Wo, bo):
    outp, _ = run(x, cos, sin, Wq, Wk, Wv, Wo, bo, t=T)
    return outp


# revision 9
# speedup vs baseline: 1.0907x; 1.0907x over previous
"""Trainium2 Bass kernel for CausalSelfAttention (GQA + QK-RMSNorm + RoPE).

Problem shapes (hardcoded): B=2, T=2048, C=2048, n_head=16, n_kv_head=4,
head_dim=128. 8 NeuronCores: data-parallel over batch (2) x tensor-parallel
over kv-head groups (4). Core j handles batch j//4 and kv-head group j%4
(q heads 4*(j%4)..4*(j%4)+3). Each core computes a partial output projection
with its Wo row-slice; the host sums the 4 partials per batch (the unshard
step) and adds the bias.

On-device dtype is fp16 (inputs pre-cast on host) with fp32 PSUM accumulation.
Layout strategy (all matmuls contract over the partition dim):
  x^T via DMA-transpose from DRAM -> q/k/v projections in natural [T, d]
  layout -> RMSNorm (Square+accum on ACT, rsqrt = exp(-0.5*ln(ms)) to stay in
  one ACT table) + RoPE on DVE -> PE-transpose to q^T/k^T (fp16 PSUM) ->
  S^T = k^T' @ q^T per tk-chunk (tq range clipped at the causal diagonal) ->
  exp on ACT (PSUM->SBUF, bias -16*ln2 keeps P in fp16 range; the uniform
  scale cancels in the softmax normalization) -> causal mask via upper-tri
  multiply on the diagonal 128x128 block -> y_aug = P^T' @ [v | 1] accumulated
  over tk (the ones column yields the softmax denominators) -> divide via ACT
  copy scaled by reciprocal -> PE-transpose y -> partial out = y^T' @ Wo.
"""
import sys
import numpy as np

for _p in ("/opt/trn_rl_repo",):
    if _p not in sys.path:
        sys.path.insert(0, _p)

import concourse.bass as bass
import concourse.bacc as bacc
import concourse.mybir as mybir
import concourse.tile as tile
from concourse.bass_utils import run_bass_kernel_spmd

F32 = mybir.dt.float32
F16 = mybir.dt.float16
AF = mybir.ActivationFunctionType
ALU = mybir.AluOpType

B, T, C = 2, 2048, 2048
N_HEAD, N_KV_HEAD = 16, 4
HD = 128           # head dim
HALF = 64
G = N_HEAD // N_KV_HEAD      # 4 q heads per kv head = heads per core
NC_ = 8
TP = 4                       # tensor-parallel width (kv heads)
QCOLS = G * HD               # 512 q columns per core
SCALE = 1.0 / float(np.sqrt(HD))
EXP_BIAS = -16.0 * float(np.log(2.0))   # keep exp() outputs inside fp16 range
EPS = float(np.finfo(np.float32).eps)

_CACHE = {}


def build_nc(t=T):
    """Build the SPMD Tile kernel for sequence length t (t % 512 == 0)."""
    nt = t // 128          # number of 128-row T-chunks
    nb = t // 512          # number of 512-wide tq blocks
    ncc = C // 128         # number of 128 C-chunks

    nc = bacc.Bacc("TRN2", target_bir_lowering=False, debug=False,
                   num_devices=NC_)

    x = nc.dram_tensor("x", [t, C], F16, kind="ExternalInput")
    wq = nc.dram_tensor("wq", [C, QCOLS], F16, kind="ExternalInput")
    wkv = nc.dram_tensor("wkv", [C, 2 * HD], F16, kind="ExternalInput")
    wo = nc.dram_tensor("wo", [QCOLS, C], F16, kind="ExternalInput")
    cs5 = nc.dram_tensor("cos5", [t, 5 * HALF], F16, kind="ExternalInput")
    sn5 = nc.dram_tensor("sin5", [t, 5 * HALF], F16, kind="ExternalInput")
    mask01 = nc.dram_tensor("mask01", [128, 128], F16, kind="ExternalInput")
    ident = nc.dram_tensor("ident16", [128, 128], F16, kind="ExternalInput")
    out = nc.dram_tensor("out", [t, C], F16, kind="ExternalOutput")

    with tile.TileContext(nc) as tc:
        with (
            tc.tile_pool(name="const", bufs=1) as cpool,
            tc.tile_pool(name="wts", bufs=1) as wpool,
            tc.tile_pool(name="big", bufs=1) as bpool,
            tc.tile_pool(name="work", bufs=3) as work,
        ):
            # ------------- constants / weights (ACT hwdge queue) -----------
            mask_t = cpool.tile([128, 128], F16, tag="mask")
            nc.scalar.dma_start(mask_t[:], mask01[:])
            id16 = cpool.tile([128, 128], F16, tag="id16")
            nc.scalar.dma_start(id16[:], ident[:])
            b_exp = cpool.tile([128, 1], F32, tag="bexp")
            nc.vector.memset(b_exp[:], EXP_BIAS)
            b_eps = cpool.tile([128, 1], F32, tag="beps")
            nc.vector.memset(b_eps[:], EPS)

            wq_t = wpool.tile([128, ncc, QCOLS], F16, tag="wq")
            nc.scalar.dma_start(
                wq_t[:], wq[:].rearrange("(cc p) n -> p cc n", p=128))
            wkv_t = wpool.tile([128, ncc, 2 * HD], F16, tag="wkv")
            nc.scalar.dma_start(
                wkv_t[:], wkv[:].rearrange("(cc p) n -> p cc n", p=128))
            wo_t = wpool.tile([128, G, C], F16, tag="wo")
            nc.scalar.dma_start(
                wo_t[:], wo[:].rearrange("(hc p) n -> p hc n", p=128))
            cos_t = wpool.tile([128, nt, 5 * HALF], F16, tag="cos")
            nc.scalar.dma_start(
                cos_t[:], cs5[:].rearrange("(i p) n -> p i n", p=128))
            sin_t = wpool.tile([128, nt, 5 * HALF], F16, tag="sin")
            nc.scalar.dma_start(
                sin_t[:], sn5[:].rearrange("(i p) n -> p i n", p=128))

            # persistent attention operands
            # qkT[:, h, :] = head h of q^T (h<4) or k^T (h==4)
            qkT = bpool.tile([128, 5, t], F16, tag="qkT")
            v_aug = bpool.tile([128, nt * (HD + 1)], F16, tag="vaug")
            nc.vector.memset(v_aug[:], 1.0)   # ones columns; data overwritten
            yT = [bpool.tile([128, t], F16, tag=f"yT{h}", name=f"yT{h}")
                  for h in range(G)]
            # raw (pre-rope) q/k in natural layout, per chunk [q0..q3 | k]
            qk_nat = [bpool.tile([128, 5 * HD], F16, tag=f"qn{i}",
                                 name=f"qn{i}") for i in range(nt)]
            sumsq_all = bpool.tile([128, nt * 5], F32, tag="ssq")
            r_all = bpool.tile([128, nt * 5], F32, tag="rall")
            rk_sc = bpool.tile([128, nt], F32, tag="rksc")

            # ---------------- phase 1a: x^T spans + projections + stats ---
            with (
                tc.tile_pool(name="ps_proj", bufs=2, space="PSUM") as ps_proj,
                tc.tile_pool(name="xsp", bufs=2) as xsp,
            ):
              for sp_i in range(t // 512):
                xTs = []
                for cc in range(ncc):
                    xt = xsp.tile([128, 512], F16, tag=f"x{cc}",
                                  name=f"x{cc}")
                    nc.sync.dma_start_transpose(
                        xt[:], x[sp_i * 512:(sp_i + 1) * 512,
                                 cc * 128:(cc + 1) * 128])
                    xTs.append(xt)
                for ii in range(4):
                    i = sp_i * 4 + ii
                    q_ps = ps_proj.tile([128, QCOLS], F32, tag="qps")
                    kv_ps = ps_proj.tile([128, 2 * HD], F32, tag="kvps")
                    for cc in range(ncc):
                        st = (cc == 0)
                        spf = (cc == ncc - 1)
                        nc.tensor.matmul(q_ps[:],
                                         xTs[cc][:, ii * 128:(ii + 1) * 128],
                                         wq_t[:, cc, :], start=st, stop=spf)
                        nc.tensor.matmul(kv_ps[:],
                                         xTs[cc][:, ii * 128:(ii + 1) * 128],
                                         wkv_t[:, cc, :], start=st, stop=spf)

                    # rms sumsq for 4 q heads + 1 k head (ACT, no table churn)
                    sq_scr = work.tile([128, 128], F16, tag="sqscr")
                    for h in range(G):
                        nc.scalar.activation(
                            sq_scr[:], q_ps[:, h * HD:(h + 1) * HD],
                            AF.Square, accum_out=sumsq_all[:, i * 5 + h:i * 5 + h + 1])
                    nc.scalar.activation(
                        sq_scr[:], kv_ps[:, 0:HD], AF.Square,
                        accum_out=sumsq_all[:, i * 5 + 4:i * 5 + 5])

                    # raw evacuation on DVE (fp16)
                    nc.vector.tensor_copy(qk_nat[i][:, 0:QCOLS], q_ps[:])
                    nc.vector.tensor_copy(qk_nat[i][:, QCOLS:5 * HD],
                                          kv_ps[:, 0:HD])
                    nc.vector.tensor_copy(
                        v_aug[:, i * (HD + 1):i * (HD + 1) + HD],
                        kv_ps[:, HD:2 * HD])

              # one Ln+Exp pair for every chunk/head: r = rsqrt(ms + eps)
              lnms = work.tile([128, nt * 5], F32, tag="lnms")
              nc.scalar.activation(lnms[:], sumsq_all[:], AF.Ln,
                                   bias=b_eps[:], scale=1.0 / HD)
              nc.scalar.activation(r_all[:], lnms[:], AF.Exp, scale=-0.5)
              rk_view = r_all[:].rearrange("p (i f) -> p i f", f=5)[:, :, 4:5]
              nc.vector.tensor_scalar(
                  rk_sc[:].rearrange("p (i f) -> p i f", f=1), rk_view,
                  SCALE, None, ALU.mult)

              # -------------- phase 1b: scale q, rope, transpose ----------
              for i in range(nt):
                for h in range(G):
                    nc.vector.tensor_scalar(
                        qk_nat[i][:, h * HD:(h + 1) * HD],
                        qk_nat[i][:, h * HD:(h + 1) * HD],
                        r_all[:, i * 5 + h:i * 5 + h + 1], None, ALU.mult)

                # rope over 5 heads at once (strided views)
                qk_r = work.tile([128, 5 * HD], F16, tag="qkr")
                lo = qk_nat[i][:].rearrange("p (h d) -> p h d", d=HD)[:, :, 0:HALF]
                hi = qk_nat[i][:].rearrange("p (h d) -> p h d", d=HD)[:, :, HALF:HD]
                lo_o = qk_r[:].rearrange("p (h d) -> p h d", d=HD)[:, :, 0:HALF]
                hi_o = qk_r[:].rearrange("p (h d) -> p h d", d=HD)[:, :, HALF:HD]
                cos_v = cos_t[:, i, :].rearrange("p (h f) -> p h f", f=HALF)
                sin_v = sin_t[:, i, :].rearrange("p (h f) -> p h f", f=HALF)
                t1 = work.tile([128, 5 * HALF], F16, tag="t1")
                t2 = work.tile([128, 5 * HALF], F16, tag="t2")
                t1v = t1[:].rearrange("p (h f) -> p h f", f=HALF)
                t2v = t2[:].rearrange("p (h f) -> p h f", f=HALF)
                nc.vector.tensor_tensor(t1v, lo, cos_v, ALU.mult)
                nc.vector.tensor_tensor(t2v, hi, sin_v, ALU.mult)
                nc.vector.tensor_tensor(lo_o, t1v, t2v, ALU.subtract)
                nc.vector.tensor_tensor(t1v, lo, sin_v, ALU.mult)
                nc.vector.tensor_tensor(t2v, hi, cos_v, ALU.mult)
                nc.vector.tensor_tensor(hi_o, t1v, t2v, ALU.add)

                # PE-transpose the 5 roped heads -> fp16 PSUM -> DVE evac
                tps = ps_proj.tile([128, 5 * HD], F16, tag="tps")
                for h in range(5):
                    nc.tensor.transpose(tps[:, h * HD:(h + 1) * HD],
                                        qk_r[:, h * HD:(h + 1) * HD], id16[:])
                nc.vector.tensor_copy(
                    qkT[:, :, i * 128:(i + 1) * 128],
                    tps[:].rearrange("p (h d) -> p h d", d=HD))

            # ---------------- phases 2+3: attention + output projection ---
            with (
                tc.tile_pool(name="ps_s", bufs=2, space="PSUM") as ps_s,
                tc.tile_pool(name="ps_y", bufs=4, space="PSUM") as ps_y,
                tc.tile_pool(name="ps_t", bufs=2, space="PSUM") as ps_t,
            ):
              for h in range(G):
                for b in range(nb):
                    y_ps = [ps_y.tile([128, HD + 1], F32, tag="y", name=f"y{j}")
                            for j in range(4)]
                    nk = 4 * (b + 1)        # causal: tk chunks 0..nk-1
                    for k in range(nk):
                        # clip the tq range at the causal diagonal
                        tq0 = max(b * 512, k * 128)
                        w = (b + 1) * 512 - tq0
                        s_ps = ps_s.tile([128, 512], F32, tag="s")
                        nc.tensor.matmul(s_ps[:, 0:w],
                                         qkT[:, 4, k * 128:(k + 1) * 128],
                                         qkT[:, h, tq0:tq0 + w],
                                         start=True, stop=True)
                        pT = work.tile([128, 512], F16, tag="pT")
                        nc.scalar.activation(pT[:, 0:w], s_ps[:, 0:w], AF.Exp,
                                             bias=b_exp[:],
                                             scale=rk_sc[:, k:k + 1])
                        if k * 128 >= b * 512:   # diagonal block at offset 0
                            nc.gpsimd.tensor_mul(
                                pT[:, 0:128], pT[:, 0:128], mask_t[:])
                        for j in range(4):
                            tq = b * 4 + j
                            if tq * 128 < tq0:   # fully masked (tq < tk)
                                continue
                            off = tq * 128 - tq0
                            nc.tensor.matmul(
                                y_ps[j][:], pT[:, off:off + 128],
                                v_aug[:, k * (HD + 1):(k + 1) * (HD + 1)],
                                start=(k == 0), stop=(k == tq),
                                skip_group_check=True)
                    # divide by softmax sums, PE-transpose, DVE evac
                    ytp = ps_t.tile([128, 512], F16, tag="ytp")
                    for j in range(4):
                        rcp = work.tile([128, 1], F32, tag="rcp")
                        nc.vector.reciprocal(rcp[:], y_ps[j][:, HD:HD + 1])
                        y_sb = work.tile([128, HD], F16, tag="ysb")
                        nc.vector.tensor_scalar(y_sb[:], y_ps[j][:, 0:HD],
                                                rcp[:], None, ALU.mult)
                        nc.tensor.transpose(ytp[:, j * 128:(j + 1) * 128],
                                            y_sb[:], id16[:])
                    nc.vector.tensor_copy(yT[h][:, b * 512:(b + 1) * 512],
                                          ytp[:])

              # -------------- output projection (partial) ------------------
              for tq in range(nt):
                for cb in range(C // 512):
                    o_ps = ps_s.tile([128, 512], F32, tag="s", name="o_ps")
                    for hc in range(G):
                        nc.tensor.matmul(o_ps[:],
                                         yT[hc][:, tq * 128:(tq + 1) * 128],
                                         wo_t[:, hc, cb * 512:(cb + 1) * 512],
                                         start=(hc == 0), stop=(hc == G - 1))
                    o_sb = work.tile([128, 512], F16, tag="osb")
                    if cb % 4 != 3:
                        nc.scalar.copy(o_sb[:], o_ps[:])
                    else:
                        nc.vector.tensor_copy(o_sb[:], o_ps[:])
                    nc.sync.dma_start(
                        out[tq * 128:(tq + 1) * 128, cb * 512:(cb + 1) * 512],
                        o_sb[:])
    nc.compile()
    return nc


def _prep_inputs(x, cos, sin, Wq, Wk, Wv, Wo, bo, t):
    """Build the 8 per-core input maps (host-side shard + fp16 cast)."""
    cos2 = np.asarray(cos, np.float32).reshape(-1, HALF)[:t]
    sin2 = np.asarray(sin, np.float32).reshape(-1, HALF)[:t]
    cos5 = np.tile(cos2, (1, 5)).astype(np.float16)
    sin5 = np.tile(sin2, (1, 5)).astype(np.float16)
    mask = np.triu(np.ones((128, 128), np.float16))  # [tk, tq]: 1 iff tq >= tk
    ident = np.eye(128, dtype=np.float16)
    Wq = np.asarray(Wq, np.float32)
    Wk = np.asarray(Wk, np.float32)
    Wv = np.asarray(Wv, np.float32)
    Wo = np.asarray(Wo, np.float32)
    x = np.asarray(x, np.float32)
    maps = []
    for core in range(NC_):
        b, tp = core // TP, core % TP
        wkv = np.concatenate(
            [Wk[:, tp * HD:(tp + 1) * HD], Wv[:, tp * HD:(tp + 1) * HD]],
            axis=1)
        maps.append({
            "x": x[b, :t].astype(np.float16),
            "wq": Wq[:, tp * QCOLS:(tp + 1) * QCOLS].astype(np.float16),
            "wkv": wkv.astype(np.float16),
            "wo": Wo[tp * QCOLS:(tp + 1) * QCOLS, :].astype(np.float16),
            "cos5": cos5, "sin5": sin5, "mask01": mask, "ident16": ident,
        })
    return maps


def run(x, cos, sin, Wq, Wk, Wv, Wo, bo, t=T, trace=False):
    key = t
    if key not in _CACHE:
        _CACHE[key] = build_nc(t)
    nc = _CACHE[key]
    maps = _prep_inputs(x, cos, sin, Wq, Wk, Wv, Wo, bo, t)
    res = run_bass_kernel_spmd(nc, maps, core_ids=list(range(NC_)),
                               trace=trace)
    bo = np.asarray(bo, np.float32)
    outp = np.empty((B, t, C), np.float32)
    for b in range(B):
        acc = res.results[b * TP]["out"].astype(np.float32)
        for tp in range(1, TP):
            acc += res.results[b * TP + tp]["out"].astype(np.float32)
        outp[b] = acc + bo[None, :]
    return outp, res


def kernel(x, cos, sin, Wq, Wk, Wv, Wo, bo):
    outp, _ = run(x, cos, sin, Wq, Wk, Wv, Wo, bo, t=T)
    return outp


# revision 10
# speedup vs baseline: 1.2815x; 1.1749x over previous
"""Trainium2 Bass kernel for CausalSelfAttention (GQA + QK-RMSNorm + RoPE).

Problem shapes (hardcoded): B=2, T=2048, C=2048, n_head=16, n_kv_head=4,
head_dim=128. 8 NeuronCores: data-parallel over batch (2) x tensor-parallel
over kv-head groups (4). Core j handles batch j//4 and kv-head group j%4
(q heads 4*(j%4)..4*(j%4)+3). Each core computes a partial output projection
with its Wo row-slice; the host sums the 4 partials per batch (the unshard
step) and adds the bias.

On-device dtype is fp16 (inputs pre-cast on host) with fp32 PSUM accumulation.
Layout strategy (all matmuls contract over the partition dim):
  x^T via DMA-transpose from DRAM -> q/k/v projections in natural [T, d]
  layout -> RMSNorm (Square+accum on ACT, rsqrt = exp(-0.5*ln(ms)) to stay in
  one ACT table) + RoPE on DVE -> PE-transpose to q^T/k^T (fp16 PSUM) ->
  S^T = k^T' @ q^T per tk-chunk (tq range clipped at the causal diagonal) ->
  exp on ACT (PSUM->SBUF, bias -16*ln2 keeps P in fp16 range; the uniform
  scale cancels in the softmax normalization) -> causal mask via upper-tri
  multiply on the diagonal 128x128 block -> y_aug = P^T' @ [v | 1] accumulated
  over tk (the ones column yields the softmax denominators) -> divide via ACT
  copy scaled by reciprocal -> PE-transpose y -> partial out = y^T' @ Wo.
"""
import sys
import numpy as np

for _p in ("/opt/trn_rl_repo",):
    if _p not in sys.path:
        sys.path.insert(0, _p)

import concourse.bass as bass
import concourse.bacc as bacc
import concourse.mybir as mybir
import concourse.tile as tile
from concourse.bass_utils import run_bass_kernel_spmd

F32 = mybir.dt.float32
F16 = mybir.dt.float16
AF = mybir.ActivationFunctionType
ALU = mybir.AluOpType

B, T, C = 2, 2048, 2048
N_HEAD, N_KV_HEAD = 16, 4
HD = 128           # head dim
HALF = 64
G = N_HEAD // N_KV_HEAD      # 4 q heads per kv head = heads per core
NC_ = 8
TP = 4                       # tensor-parallel width (kv heads)
QCOLS = G * HD               # 512 q columns per core
SCALE = 1.0 / float(np.sqrt(HD))
EXP_BIAS = -16.0 * float(np.log(2.0))   # keep exp() outputs inside fp16 range
EPS = float(np.finfo(np.float32).eps)

_CACHE = {}


def build_nc(t=T):
    """Build the SPMD Tile kernel for sequence length t (t % 512 == 0)."""
    nt = t // 128          # number of 128-row T-chunks
    nb = t // 512          # number of 512-wide tq blocks
    ncc = C // 128         # number of 128 C-chunks

    nc = bacc.Bacc("TRN2", target_bir_lowering=False, debug=False,
                   num_devices=NC_)

    xtd = nc.dram_tensor("xT", [C, t], F16, kind="ExternalInput")
    wq = nc.dram_tensor("wq", [C, QCOLS], F16, kind="ExternalInput")
    wkv = nc.dram_tensor("wkv", [C, 2 * HD], F16, kind="ExternalInput")
    wo = nc.dram_tensor("wo", [QCOLS, C], F16, kind="ExternalInput")
    cs5 = nc.dram_tensor("cos5", [t, 5 * HALF], F16, kind="ExternalInput")
    sn5 = nc.dram_tensor("sin5", [t, 5 * HALF], F16, kind="ExternalInput")
    mask01 = nc.dram_tensor("mask01", [128, 128], F16, kind="ExternalInput")
    ident = nc.dram_tensor("ident16", [128, 128], F16, kind="ExternalInput")
    out = nc.dram_tensor("out", [t, C], F16, kind="ExternalOutput")

    with tile.TileContext(nc) as tc:
        with (
            tc.tile_pool(name="const", bufs=1) as cpool,
            tc.tile_pool(name="wts", bufs=1) as wpool,
            tc.tile_pool(name="big", bufs=1) as bpool,
            tc.tile_pool(name="work", bufs=3) as work,
        ):
            # ------------- constants / weights (ACT hwdge queue) -----------
            mask_t = cpool.tile([128, 128], F16, tag="mask")
            nc.scalar.dma_start(mask_t[:], mask01[:])
            id16 = cpool.tile([128, 128], F16, tag="id16")
            nc.scalar.dma_start(id16[:], ident[:])
            b_exp = cpool.tile([128, 1], F32, tag="bexp")
            nc.vector.memset(b_exp[:], EXP_BIAS)
            b_eps = cpool.tile([128, 1], F32, tag="beps")
            nc.vector.memset(b_eps[:], EPS)

            wq_t = wpool.tile([128, ncc, QCOLS], F16, tag="wq")
            nc.scalar.dma_start(
                wq_t[:], wq[:].rearrange("(cc p) n -> p cc n", p=128))
            wkv_t = wpool.tile([128, ncc, 2 * HD], F16, tag="wkv")
            nc.scalar.dma_start(
                wkv_t[:], wkv[:].rearrange("(cc p) n -> p cc n", p=128))
            wo_t = wpool.tile([128, G, C], F16, tag="wo")
            nc.scalar.dma_start(
                wo_t[:], wo[:].rearrange("(hc p) n -> p hc n", p=128))
            cos_t = wpool.tile([128, nt, 5 * HALF], F16, tag="cos")
            nc.scalar.dma_start(
                cos_t[:], cs5[:].rearrange("(i p) n -> p i n", p=128))
            sin_t = wpool.tile([128, nt, 5 * HALF], F16, tag="sin")
            nc.scalar.dma_start(
                sin_t[:], sn5[:].rearrange("(i p) n -> p i n", p=128))

            # persistent attention operands
            # qkT[:, h, :] = head h of q^T (h<4) or k^T (h==4)
            qkT = bpool.tile([128, 5, t], F16, tag="qkT")
            v_aug = bpool.tile([128, nt * (HD + 1)], F16, tag="vaug")
            nc.vector.memset(v_aug[:], 1.0)   # ones columns; data overwritten
            yT = [bpool.tile([128, t], F16, tag=f"yT{h}", name=f"yT{h}")
                  for h in range(G)]
            # raw (pre-rope) q/k in natural layout, per chunk [q0..q3 | k]
            qk_nat = [bpool.tile([128, 5 * HD], F16, tag=f"qn{i}",
                                 name=f"qn{i}") for i in range(nt)]
            sumsq_all = bpool.tile([128, nt * 5], F32, tag="ssq")
            r_all = bpool.tile([128, nt * 5], F32, tag="rall")

            # ------- phase 1: per 512-span: load x^T, project, norm, rope --
            with (
                tc.tile_pool(name="ps_proj", bufs=2, space="PSUM") as ps_proj,
                tc.tile_pool(name="xsp", bufs=2) as xsp,
            ):
              for sp_i in range(t // 512):
                xTs = []
                for cc in range(ncc):
                    xt = xsp.tile([128, 512], F16, tag=f"x{cc}",
                                  name=f"x{cc}")
                    nc.sync.dma_start(
                        xt[:], xtd[cc * 128:(cc + 1) * 128,
                                   sp_i * 512:(sp_i + 1) * 512])
                    xTs.append(xt)
                for ii in range(4):
                    i = sp_i * 4 + ii
                    q_ps = ps_proj.tile([128, QCOLS], F32, tag="qps")
                    kv_ps = ps_proj.tile([128, 2 * HD], F32, tag="kvps")
                    for cc in range(ncc):
                        st = (cc == 0)
                        spf = (cc == ncc - 1)
                        nc.tensor.matmul(q_ps[:],
                                         xTs[cc][:, ii * 128:(ii + 1) * 128],
                                         wq_t[:, cc, :], start=st, stop=spf)
                        nc.tensor.matmul(kv_ps[:],
                                         xTs[cc][:, ii * 128:(ii + 1) * 128],
                                         wkv_t[:, cc, :], start=st, stop=spf)

                    # rms sumsq for 4 q heads + 1 k head (ACT, no table churn)
                    sq_scr = work.tile([128, 128], F16, tag="sqscr")
                    for h in range(G):
                        nc.scalar.activation(
                            sq_scr[:], q_ps[:, h * HD:(h + 1) * HD],
                            AF.Square, accum_out=sumsq_all[:, i * 5 + h:i * 5 + h + 1])
                    nc.scalar.activation(
                        sq_scr[:], kv_ps[:, 0:HD], AF.Square,
                        accum_out=sumsq_all[:, i * 5 + 4:i * 5 + 5])

                    # raw evacuation on DVE (fp16)
                    nc.vector.tensor_copy(qk_nat[i][:, 0:QCOLS], q_ps[:])
                    nc.vector.tensor_copy(qk_nat[i][:, QCOLS:5 * HD],
                                          kv_ps[:, 0:HD])
                    nc.vector.tensor_copy(
                        v_aug[:, i * (HD + 1):i * (HD + 1) + HD],
                        kv_ps[:, HD:2 * HD])

                # span r-chain: r = rsqrt(ms + eps) = exp(-0.5*ln(ms + eps))
                c0, c1 = sp_i * 20, (sp_i + 1) * 20
                lnms = work.tile([128, 20], F32, tag="lnms")
                nc.scalar.activation(lnms[:], sumsq_all[:, c0:c1], AF.Ln,
                                     bias=b_eps[:], scale=1.0 / HD)
                nc.scalar.activation(r_all[:, c0:c1], lnms[:], AF.Exp,
                                     scale=-0.5)

                # phase 1b for this span: scale q (r) and k (r*softmax scale),
                # rope, PE-transpose into q^T/k^T
                for ii in range(4):
                    i = sp_i * 4 + ii
                    for h in range(G):
                        nc.vector.tensor_scalar(
                            qk_nat[i][:, h * HD:(h + 1) * HD],
                            qk_nat[i][:, h * HD:(h + 1) * HD],
                            r_all[:, i * 5 + h:i * 5 + h + 1], None, ALU.mult)
                    nc.vector.tensor_scalar(
                        qk_nat[i][:, 4 * HD:5 * HD],
                        qk_nat[i][:, 4 * HD:5 * HD],
                        r_all[:, i * 5 + 4:i * 5 + 5], SCALE,
                        ALU.mult, ALU.mult)

                    qk_r = work.tile([128, 5 * HD], F16, tag="qkr")
                    lo = qk_nat[i][:].rearrange("p (h d) -> p h d", d=HD)[:, :, 0:HALF]
                    hi = qk_nat[i][:].rearrange("p (h d) -> p h d", d=HD)[:, :, HALF:HD]
                    lo_o = qk_r[:].rearrange("p (h d) -> p h d", d=HD)[:, :, 0:HALF]
                    hi_o = qk_r[:].rearrange("p (h d) -> p h d", d=HD)[:, :, HALF:HD]
                    cos_v = cos_t[:, i, :].rearrange("p (h f) -> p h f", f=HALF)
                    sin_v = sin_t[:, i, :].rearrange("p (h f) -> p h f", f=HALF)
                    t1 = work.tile([128, 5 * HALF], F16, tag="t1")
                    t2 = work.tile([128, 5 * HALF], F16, tag="t2")
                    t1v = t1[:].rearrange("p (h f) -> p h f", f=HALF)
                    t2v = t2[:].rearrange("p (h f) -> p h f", f=HALF)
                    nc.vector.tensor_tensor(t1v, lo, cos_v, ALU.mult)
                    nc.vector.tensor_tensor(t2v, hi, sin_v, ALU.mult)
                    nc.vector.tensor_tensor(lo_o, t1v, t2v, ALU.subtract)
                    nc.vector.tensor_tensor(t1v, lo, sin_v, ALU.mult)
                    nc.vector.tensor_tensor(t2v, hi, cos_v, ALU.mult)
                    nc.vector.tensor_tensor(hi_o, t1v, t2v, ALU.add)

                    tps = ps_proj.tile([128, 5 * HD], F16, tag="tps")
                    for h in range(5):
                        nc.tensor.transpose(tps[:, h * HD:(h + 1) * HD],
                                            qk_r[:, h * HD:(h + 1) * HD],
                                            id16[:])
                    nc.vector.tensor_copy(
                        qkT[:, :, i * 128:(i + 1) * 128],
                        tps[:].rearrange("p (h d) -> p h d", d=HD))

            # ---------------- phases 2+3: attention + output projection ---
            with (
                tc.tile_pool(name="ps_s", bufs=2, space="PSUM") as ps_s,
                tc.tile_pool(name="ps_y", bufs=4, space="PSUM") as ps_y,
                tc.tile_pool(name="ps_t", bufs=2, space="PSUM") as ps_t,
            ):
              for h in range(G):
                for b in range(nb):
                    y_ps = [ps_y.tile([128, HD + 1], F32, tag="y", name=f"y{j}")
                            for j in range(4)]
                    nk = 4 * (b + 1)        # causal: tk chunks 0..nk-1
                    for k in range(nk):
                        # clip the tq range at the causal diagonal
                        tq0 = max(b * 512, k * 128)
                        w = (b + 1) * 512 - tq0
                        s_ps = ps_s.tile([128, 512], F32, tag="s")
                        nc.tensor.matmul(s_ps[:, 0:w],
                                         qkT[:, 4, k * 128:(k + 1) * 128],
                                         qkT[:, h, tq0:tq0 + w],
                                         start=True, stop=True)
                        pT = work.tile([128, 512], F16, tag="pT")
                        nc.scalar.activation(pT[:, 0:w], s_ps[:, 0:w], AF.Exp,
                                             bias=b_exp[:], scale=1.0)
                        if k * 128 >= b * 512:   # diagonal block at offset 0
                            nc.gpsimd.tensor_mul(
                                pT[:, 0:128], pT[:, 0:128], mask_t[:])
                        for j in range(4):
                            tq = b * 4 + j
                            if tq * 128 < tq0:   # fully masked (tq < tk)
                                continue
                            off = tq * 128 - tq0
                            nc.tensor.matmul(
                                y_ps[j][:], pT[:, off:off + 128],
                                v_aug[:, k * (HD + 1):(k + 1) * (HD + 1)],
                                start=(k == 0), stop=(k == tq),
                                skip_group_check=True)
                    # divide by softmax sums, PE-transpose, DVE evac
                    ytp = ps_t.tile([128, 512], F16, tag="ytp")
                    for j in range(4):
                        rcp = work.tile([128, 1], F32, tag="rcp")
                        nc.vector.reciprocal(rcp[:], y_ps[j][:, HD:HD + 1])
                        y_sb = work.tile([128, HD], F16, tag="ysb")
                        nc.vector.tensor_scalar(y_sb[:], y_ps[j][:, 0:HD],
                                                rcp[:], None, ALU.mult)
                        nc.tensor.transpose(ytp[:, j * 128:(j + 1) * 128],
                                            y_sb[:], id16[:])
                    nc.vector.tensor_copy(yT[h][:, b * 512:(b + 1) * 512],
                                          ytp[:])

              # -------------- output projection (partial) ------------------
              for tq in range(nt):
                for cb in range(C // 512):
                    o_ps = ps_s.tile([128, 512], F32, tag="s", name="o_ps")
                    for hc in range(G):
                        nc.tensor.matmul(o_ps[:],
                                         yT[hc][:, tq * 128:(tq + 1) * 128],
                                         wo_t[:, hc, cb * 512:(cb + 1) * 512],
                                         start=(hc == 0), stop=(hc == G - 1))
                    o_sb = work.tile([128, 512], F16, tag="osb")
                    if cb % 4 != 3:
                        nc.scalar.copy(o_sb[:], o_ps[:])
                    else:
                        nc.vector.tensor_copy(o_sb[:], o_ps[:])
                    nc.sync.dma_start(
                        out[tq * 128:(tq + 1) * 128, cb * 512:(cb + 1) * 512],
                        o_sb[:])
    nc.compile()
    return nc


def _prep_inputs(x, cos, sin, Wq, Wk, Wv, Wo, bo, t):
    """Build the 8 per-core input maps (host-side shard + fp16 cast)."""
    cos2 = np.asarray(cos, np.float32).reshape(-1, HALF)[:t]
    sin2 = np.asarray(sin, np.float32).reshape(-1, HALF)[:t]
    cos5 = np.tile(cos2, (1, 5)).astype(np.float16)
    sin5 = np.tile(sin2, (1, 5)).astype(np.float16)
    mask = np.triu(np.ones((128, 128), np.float16))  # [tk, tq]: 1 iff tq >= tk
    ident = np.eye(128, dtype=np.float16)
    Wq = np.asarray(Wq, np.float32)
    Wk = np.asarray(Wk, np.float32)
    Wv = np.asarray(Wv, np.float32)
    Wo = np.asarray(Wo, np.float32)
    x = np.asarray(x, np.float32)
    maps = []
    for core in range(NC_):
        b, tp = core // TP, core % TP
        wkv = np.concatenate(
            [Wk[:, tp * HD:(tp + 1) * HD], Wv[:, tp * HD:(tp + 1) * HD]],
            axis=1)
        maps.append({
            "xT": np.ascontiguousarray(x[b, :t].astype(np.float16).T),
            "wq": Wq[:, tp * QCOLS:(tp + 1) * QCOLS].astype(np.float16),
            "wkv": wkv.astype(np.float16),
            "wo": Wo[tp * QCOLS:(tp + 1) * QCOLS, :].astype(np.float16),
            "cos5": cos5, "sin5": sin5, "mask01": mask, "ident16": ident,
        })
    return maps


def run(x, cos, sin, Wq, Wk, Wv, Wo, bo, t=T, trace=False):
    key = t
    if key not in _CACHE:
        _CACHE[key] = build_nc(t)
    nc = _CACHE[key]
    maps = _prep_inputs(x, cos, sin, Wq, Wk, Wv, Wo, bo, t)
    res = run_bass_kernel_spmd(nc, maps, core_ids=list(range(NC_)),
                               trace=trace)
    bo = np.asarray(bo, np.float32)
    outp = np.empty((B, t, C), np.float32)
    for b in range(B):
        acc = res.results[b * TP]["out"].astype(np.float32)
        for tp in range(1, TP):
            acc += res.results[b * TP + tp]["out"].astype(np.float32)
        outp[b] = acc + bo[None, :]
    return outp, res


def kernel(x, cos, sin, Wq, Wk, Wv, Wo, bo):
    outp, _ = run(x, cos, sin, Wq, Wk, Wv, Wo, bo, t=T)
    return outp
